# revision 31
# baseline (speedup 1.0000x reference)
"""Trainium2 Bass kernel for fused segment-mean + linear projection.

Reference computation (for x[N,15], sorted batch[N] in [0,G), W[5,15], b[5]):
    sums[g]  = segment_sum(x, batch)          # [G, 15]
    counts[g]= segment_sum(1, batch)          # [G]
    mean     = sums / max(counts, 1)
    out      = where(counts > 0, mean @ W.T + b, 0)   # [G, 5]

Strategy (8 NeuronCores, data parallel over contiguous graph-id ranges):
  Host (index-only preprocessing; x is repacked and cast to bf16 --
  the 2e-2 correctness budget dwarfs bf16 quantization error, and it
  halves the HBM traffic this memory-bound kernel is limited by):
    - each core owns G/8 consecutive graphs; its nodes are repacked into
      "windows" of GPW=32 graphs x 8192 node-slots (4 chunks of 128
      16-node blocks), each graph zero-padded to a 16-node multiple so
      every block belongs to exactly one graph. Graphs that do not fit
      their window spill whole into a small per-stripe overflow stream.
    - the packed stream is bf16 PARTITION-MAJOR: each SBUF partition's
      chunk range is contiguous in DRAM, so DMA descriptors stay >= 512B
      (full 360 GB/s; no small-descriptor penalty) at half the bytes.
    - all program shapes / the matmul schedule are data-independent, so
      one SPMD program serves all 8 cores; per-core data differs only in
      the input tables.
  Device (per core):
    - DMA bf16 x tiles (tapered sizes); block sums via a 4-stage DVE
      tensor_tensor halving tree (16->8->4->2->1 nodes). Blocks are
      node-major so every stage adds two contiguous 15-element runs:
      all operands are 2-byte stride-1 -> DVE 2x mode on every stage
      (tensor_reduce has no fast modes, hence the tt tree).
    - PE bf16 matmuls with one-hot matrices scatter-add block sums
      into per-stripe-group PSUM accumulators (f32, exact); the one-hot
      arenas are built by GPSIMD local_scatter from host-made int16
      index tables (zero DVE cost); per-group start/stop lets each
      group flush mid-stream.
    - per group: one copy flushes PSUM -> bf16 (ACT mid-stream; DVE
      for the last two groups, cutting cross-engine hops off the tail),
      a PE identity matmul recombines the four 32-graph quadrants into
      128 rows, and DVE applies the fused winv = W * (1/count) table,
      reduces over features, adds bias * nonempty -> out rows; the
      1-stripe final group keeps the post-stream tail short.
  Host: concatenate the 8 core outputs.
"""

import sys

for _p in ("/opt/trn_rl_repo",):
    if _p not in sys.path:
        sys.path.insert(0, _p)

import numpy as np
import ml_dtypes
from contextlib import ExitStack

import concourse.bass as bass
import concourse.bacc as bacc
import concourse.tile as tile
from concourse import mybir
from concourse.bass_utils import run_bass_kernel_spmd

P = 128          # partitions
BLK = 16         # nodes per block
D = 15           # feature dim
O = 5            # output dim
GPW = 32         # graphs per window
CPW = 4          # chunks per window (chunk = 128 blocks = 2048 node slots)
SLOTS_W = CPW * P * BLK  # node slots per window

F32 = mybir.dt.float32
BF16 = mybir.dt.bfloat16
BNP = ml_dtypes.bfloat16


# ----------------------------------------------------------------------------
# host planner
# ----------------------------------------------------------------------------

class Plan:
    """Per-run packing plan. All *shape* fields are uniform across cores."""

    def __init__(self, batch, n_cores, G, W=None, b=None):
        self.W = (np.zeros((O, D), np.float32) if W is None
                  else np.asarray(W, np.float32))
        self.b = (np.zeros(O, np.float32) if b is None
                  else np.asarray(b, np.float32))
        batch = np.asarray(batch)
        N = batch.shape[0]
        assert G % (n_cores * P) == 0
        self.G = G
        self.n_cores = n_cores
        self.gpc = G // n_cores                  # graphs per core
        self.nwin = self.gpc // GPW              # windows per core
        self.nstripe = self.gpc // P             # psum stripes per core
        self.nchunk = self.nwin * CPW            # main-stream chunks per core
        self.lslots = self.nwin * SLOTS_W        # node slots per core
        assert self.nwin % self.nstripe == 0
        self.wps = self.nwin // self.nstripe     # windows per stripe
        self.cps = self.nchunk // self.nstripe   # chunks per stripe

        bounds = np.searchsorted(batch, np.arange(G + 1))
        counts = np.diff(bounds).astype(np.int64)
        self.counts = counts
        self.inv = (1.0 / np.maximum(counts, 1.0)).astype(np.float32)
        self.nonempty = (counts > 0).astype(np.float32)

        nblk_g = (counts + BLK - 1) // BLK       # blocks per graph

        # ---- window placement (per core) ----
        self.placements = []
        self.overflow = []
        max_oslots = 8 * BLK
        for c in range(n_cores):
            g0 = c * self.gpc
            placed = []
            oflow = [[] for _ in range(self.nstripe)]
            for w in range(self.nwin):
                gs = [g0 + w * GPW + j for j in range(GPW)
                      if nblk_g[g0 + w * GPW + j] > 0]
                need = {g: int(nblk_g[g]) * BLK for g in gs}
                over = sum(need.values()) - SLOTS_W
                spill = []
                while over > 0:
                    # spill the smallest graph that covers the overage, or
                    # the largest graph if none does (minimizes spilled slots)
                    cand = [g for g in gs if need[g] >= over]
                    g = (min(cand, key=need.get) if cand
                         else max(gs, key=need.get))
                    gs.remove(g); spill.append(g); over -= need[g]
                pos = 0
                for g in gs:
                    placed.append((g, w * SLOTS_W + pos))
                    pos += need[g]
                oflow[w // self.wps].extend(spill)
            for s in range(self.nstripe):
                used = sum(int(nblk_g[g]) * BLK for g in oflow[s])
                max_oslots = max(max_oslots, used)
            self.placements.append(placed)
            self.overflow.append(oflow)

        # overflow blocks per stripe: a single partial chunk (K<128 matmuls)
        # when it fits, whole chunks otherwise
        max_oblk = -(-max_oslots // BLK)
        if max_oblk <= P:
            self.osb = max(8, -(-max_oblk // 8) * 8)
            self.ocps = 1
        else:
            self.osb = P
            self.ocps = -(-max_oblk // P)
        self.osps = self.ocps * self.osb * BLK   # overflow slots per stripe
        self.noch = self.nstripe * self.ocps     # total overflow chunks
        self.bounds = bounds
        self.N = N

    def core_tables(self, c, x):
        """Build per-core input arrays. x is the full [N, D] float32 array."""
        lslots, nchunk = self.lslots, self.nchunk
        g0 = c * self.gpc

        idx = np.full(lslots, -1, dtype=np.int64)
        asg = np.full(lslots // BLK, -1.0, dtype=np.float32)  # local graph/blk
        for g, base in self.placements[c]:
            s0, cnt = int(self.bounds[g]), int(self.counts[g])
            idx[base : base + cnt] = np.arange(s0, s0 + cnt)
            nb = (cnt + BLK - 1) // BLK
            asg[base // BLK : base // BLK + nb] = g - g0

        # overflow stream: per stripe a fixed region of osps slots
        oidx = np.full(self.nstripe * self.osps, -1, dtype=np.int64)
        oasg = np.full(self.nstripe * self.osps // BLK, -1.0, dtype=np.float32)
        for s in range(self.nstripe):
            pos = s * self.osps
            for g in self.overflow[c][s]:
                s0, cnt = int(self.bounds[g]), int(self.counts[g])
                nb = (cnt + BLK - 1) // BLK
                assert pos + nb * BLK <= (s + 1) * self.osps, "overflow overrun"
                oidx[pos : pos + cnt] = np.arange(s0, s0 + cnt)
                oasg[pos // BLK : pos // BLK + nb] = (g - g0) - s * P
                pos += nb * BLK

        def pack(idx_arr, nch, parts):
            # gather, then block t -> (partition t % parts, chunk t // parts),
            # partition-major layout, blocks in natural node-major order
            # (so every halving-tree stage is stride-1 innermost -> DVE 2x)
            out = x[np.clip(idx_arr, 0, self.N - 1)]
            out[idx_arr < 0] = 0.0
            out = out.reshape(nch, parts, BLK * D).transpose(1, 0, 2)
            return np.ascontiguousarray(out.reshape(parts, nch * BLK * D)
                                        .astype(BNP))

        xw = pack(idx, nchunk, P)                 # [P, nchunk*CB] bf16
        xb = pack(oidx, self.noch, self.osb)      # [osb, noch*CB] bf16

        # block t -> partition t%128, chunk t//128; window of chunk m = m//CPW
        # idxJ[p, m]: scatter index into the one-hot arena (w-major per
        # stripe, 1024-column sections of 2 stripes), -1 for padding blocks
        t = np.arange(lslots // BLK)
        win_base = (t // P // CPW) * GPW
        w = np.where(asg >= 0, asg - win_base, -1.0).astype(np.int64)
        m = t // P
        s = m // self.cps
        iJ = np.where(w >= 0,
                      (s % 2) * (self.cps * GPW) + w * self.cps + m % self.cps,
                      -1)
        idxJ = np.full((P, nchunk), -1, dtype=np.int16)
        idxJ[t % P, m] = iJ.astype(np.int16)

        # idxO[p, sec*noch + ch]: index into the overflow one-hot arena
        # (w*noch + ch layout, 1024-column sections of wspan=1024//noch)
        oasgT = np.full((P, self.noch), -1, dtype=np.int64)
        oasgT[: self.osb, :] = oasg.reshape(self.noch, self.osb).T.astype(
            np.int64)
        wspan = 1024 // self.noch
        nsec = P // wspan
        ch = np.arange(self.noch)[None, :]
        idxO = np.full((P, nsec * self.noch), -1, dtype=np.int16)
        for sec in range(nsec):
            wl = oasgT - sec * wspan
            ok = (wl >= 0) & (wl < wspan)
            idxO[:, sec * self.noch : (sec + 1) * self.noch] = np.where(
                ok, wl * self.noch + ch, -1).astype(np.int16)

        def stripe_pack(v):
            # graph g (local) -> [partition g%128, col g//128]
            return np.ascontiguousarray(
                v[g0 : g0 + self.gpc].reshape(self.nstripe, P).T.astype(np.float32)
            )

        inv_ps = stripe_pack(self.inv)                       # [P, nstripe] f32
        # fused W x 1/count table: winv[p, o, s, f] = W[o, f] * inv[g(p, s)]
        winv = (inv_ps[:, None, :, None] *
                self.W[None, :, None, :]).astype(BNP)        # [P,O,S,D] bf16
        bne = (stripe_pack(self.nonempty)[:, :, None] *
               self.b[None, None, :]).reshape(P, self.nstripe * O)
        # pad bne to 128 f32 cols so the DMA descriptor is >= 512B
        bne_pad = np.zeros((P, 128), np.float32)
        bne_pad[:, : self.nstripe * O] = bne
        # identity selection matrix for the quadrant recombine:
        # i4[k, q*P + m] = 1.0 iff m == q*GPW + k   (bf16, built on host)
        nq = P // GPW
        i4 = np.zeros((GPW, nq * P), BNP)
        for q in range(nq):
            for k in range(GPW):
                i4[k, q * P + q * GPW + k] = 1.0

        return {
            "xw": xw.reshape(-1),
            "xb": xb.reshape(-1),
            "idxJ": idxJ,
            "idxO": idxO,
            "winv": np.ascontiguousarray(winv.reshape(P, -1)),
            "bne": bne_pad,
            "i4": i4,
        }


# ----------------------------------------------------------------------------
# device program
# ----------------------------------------------------------------------------

def build_program(plan, W, b):
    """Build + compile the SPMD Bass program (one program, 8 cores)."""
    nchunk, noch, nstripe = plan.nchunk, plan.noch, plan.nstripe
    cps, wps, osb = plan.cps, plan.wps, plan.osb
    CB = BLK * D     # elements per block (240)

    nc = bacc.Bacc("TRN2", target_bir_lowering=False, debug=False)

    xw = nc.dram_tensor("xw", [P * nchunk * CB], BF16, kind="ExternalInput")
    xb = nc.dram_tensor("xb", [osb * noch * CB], BF16, kind="ExternalInput")
    nsec_o = P // (1024 // noch)
    idxJ = nc.dram_tensor("idxJ", [P, nchunk], mybir.dt.int16,
                          kind="ExternalInput")
    idxO = nc.dram_tensor("idxO", [P, nsec_o * noch], mybir.dt.int16,
                          kind="ExternalInput")
    winv_t = nc.dram_tensor("winv", [P, O * nstripe * D], BF16,
                            kind="ExternalInput")
    bne_t = nc.dram_tensor("bne", [P, 128], F32, kind="ExternalInput")
    i4_t = nc.dram_tensor("i4", [GPW, (P // GPW) * P], BF16,
                          kind="ExternalInput")
    out_t = nc.dram_tensor("out", [plan.gpc * O], F32, kind="ExternalOutput")

    # x tiles: chunks per DMA tile. Tapered: small first tiles so DVE starts
    # early and is never starved, small last tiles so the post-DMA tail is
    # short. Stage-1 of the big middle tiles runs on the (otherwise idle)
    # GPSIMD engine to keep the DVE chain under the DMA roofline.
    KCS = []
    rem = nchunk
    TAIL = (16, 12, 8, 4, 2)
    for k in (2, 4, 8, 8, 16, 16):
        kc = min(k, rem)
        if kc:
            KCS.append(kc); rem -= kc
    while rem > sum(TAIL):
        KCS.append(min(32, rem - sum(TAIL))); rem -= KCS[-1]
    for k in TAIL:
        kc = min(k, rem)
        if kc:
            KCS.append(kc); rem -= kc
    assert sum(KCS) == nchunk and rem == 0
    nquad = P // GPW
    # flush/recombine/projection groups of stripes; the trailing 3+1 split
    # keeps the very last group (one stripe) tiny so the tail is short.
    GROUPS = []
    s = 0
    while s + 4 < nstripe:
        GROUPS.append((s, 4)); s += 4
    if nstripe - s > 1:
        GROUPS.append((s, nstripe - s - 1)); s = nstripe - 1
    GROUPS.append((s, 1))

    with tile.TileContext(nc) as tc, ExitStack() as ctx:
        consts = ctx.enter_context(tc.tile_pool(name="consts", bufs=1))
        xpool = ctx.enter_context(tc.tile_pool(name="xpool", bufs=5))
        spool = ctx.enter_context(tc.tile_pool(name="spool", bufs=2))
        bpool = ctx.enter_context(tc.tile_pool(name="bpool", bufs=1))
        ppool = ctx.enter_context(tc.tile_pool(name="ppool", bufs=2, space="PSUM"))

        def ap_of(handle, offset, pattern):
            return bass.AP(tensor=handle.ap().tensor, offset=offset, ap=pattern)

        def tap(t, offset, pattern):
            return bass.AP(tensor=t.tensor, offset=t.offset + offset, ap=pattern)

        # ---- constant tables (ACT HWDGE ring; keeps SP ring free for x) ----
        # Only asgJ is needed immediately (first one-hot builds); the rest is
        # pinned behind the early x tiles so the x stream owns the DMA bus.
        idxJ_sb = consts.tile([P, nchunk], mybir.dt.int16)
        idxO_sb = consts.tile([P, nsec_o * noch], mybir.dt.int16)
        i4_sb = consts.tile([GPW, nquad * P], BF16)
        winv_sb = consts.tile([P, O * nstripe * D], BF16)
        bne_sb = consts.tile([P, 128], F32)

        # all-ones data row for the one-hot scatters
        ones_sb = consts.tile([P, cps * 2], BF16)
        nc.gpsimd.memset(ones_sb[:], 1.0)

        # one-hot arenas, built by GPSIMD local_scatter (dst is zeroed by
        # the instruction itself; negative indices = padding are ignored):
        #   onehot[p, s*cps*GPW + w*cps + ml] = 1 at idxJ positions
        #   oneO[p, w*noch + ch]              = 1 at idxO positions
        onehot = bpool.tile([P, nchunk * GPW], BF16)
        oneO = bpool.tile([P, P * noch], BF16)
        SEC = 2 * cps * GPW          # columns per scatter section (2 stripes)
        assert SEC == 1024 and SEC * 32 < 2 ** 16

        def emit_oh(sec):
            nix = 2 * cps            # chunks per section
            return nc.gpsimd.local_scatter(
                out_ap=onehot[:, sec * SEC : (sec + 1) * SEC],
                data_ap=ones_sb[:, :nix],
                idxs_ap=idxJ_sb[:, sec * nix : (sec + 1) * nix],
                channels=P,
                num_elems=SEC,
                num_idxs=nix,
            )

        def emit_oo(sec):
            return nc.gpsimd.local_scatter(
                out_ap=oneO[:, sec * 1024 : (sec + 1) * 1024],
                data_ap=ones_sb[:, :noch],
                idxs_ap=idxO_sb[:, sec * noch : (sec + 1) * noch],
                channels=P,
                num_elems=1024,
                num_idxs=noch,
            )

        # ---- block-sum halving tree (DVE tensor_tensor, 2x mode) ----
        # src layout per chunk-column: [D, W] feature-major, W nodes.
        def emit_tree(src, soff, dst, doff, kc, parts, tag):
            # node-major halving tree: every stage adds two contiguous
            # 15-element runs (stride-1 innermost on all operands -> 2x)
            cur, co, w = src, soff, BLK
            last = None
            while w > 1:
                h = w // 2
                if h > 1:
                    nxt = spool.tile([P, kc * D * h], BF16,
                                     tag=f"{tag}{h}", name=f"{tag}{h}")
                    no = 0
                else:
                    nxt, no = dst, doff
                cp = [cur.ap[0][0], parts]
                last = nc.vector.tensor_tensor(
                    out=tap(nxt, no, [[nxt.ap[0][0], parts],
                                      [D * h, kc], [D, h], [1, D]]),
                    in0=tap(cur, co, [cp, [D * w, kc], [D, h], [1, D]]),
                    in1=tap(cur, co + h * D,
                            [cp, [D * w, kc], [D, h], [1, D]]),
                    op=mybir.AluOpType.add,
                )
                cur, co, w = nxt, no, h
            return last

        # ---- overflow stream ----
        xb_sb = bpool.tile([P, noch * CB], BF16)
        Bo = bpool.tile([P, noch * D], BF16)

        # ---- main stream: tapered tiles -> block sums B ----
        B = bpool.tile([P, nchunk * D], BF16)
        KCMAX = max(KCS)
        c0 = 0
        oh_next = 0
        reds = []
        dmas = []
        for ti, KC in enumerate(KCS):
            xt = xpool.tile([P, KCMAX * CB], BF16, tag="xt", name="xt")
            xdma = nc.sync.dma_start(
                out=xt[:, : KC * CB],
                in_=ap_of(xw, c0 * CB, [[nchunk * CB, P], [1, KC * CB]]),
            )
            dmas.append(xdma)
            red = emit_tree(xt, 0, B, c0 * D, KC, P, "st")
            c0 += KC
            reds.append(red)
            # secondary table loads, pinned behind the early x tiles so the
            # x stream owns the DMA bus at the start
            if ti == 0:
                for dma in (
                    nc.scalar.dma_start(out=idxJ_sb[:], in_=idxJ.ap()),
                    nc.scalar.dma_start(out=idxO_sb[:], in_=idxO.ap()),
                    nc.scalar.dma_start(
                        out=xb_sb[:osb, :],
                        in_=ap_of(xb, 0, [[noch * CB, osb], [1, noch * CB]]),
                    ),
                ):
                    tile.add_dep_helper(dma.ins, dmas[0].ins, sync=False,
                                        reason="table loads behind x tiles")
            if ti == 3:
                for dma in (
                    nc.scalar.dma_start(out=i4_sb[:], in_=i4_t.ap()),
                    nc.scalar.dma_start(out=winv_sb[:], in_=winv_t.ap()),
                    nc.scalar.dma_start(out=bne_sb[:], in_=bne_t.ap()),
                ):
                    tile.add_dep_helper(dma.ins, dmas[3].ins, sync=False,
                                        reason="table loads behind x tiles")
            # Pack the small side-jobs behind the early tiles' tree work so
            # the scheduler cannot hoist them ahead of the x pipeline.
            if ti == 0:
                for sec in range(nstripe // 2):
                    emit_oh(sec)
                for sec in range(nsec_o):
                    emit_oo(sec)
            if ti == min(2, len(KCS) - 1):
                bo = emit_tree(xb_sb, 0, Bo, 0, noch, osb, "ob")
                tile.add_dep_helper(bo.ins, red.ins, sync=False,
                                    reason="Bo tree in early DVE idle")

        # ---- routing matmuls, grouped by stripe-group ----
        # Each group accumulates its stripes into ONE combined quadrant PSUM
        # tile [GPW, nquad*qs*D] (single bank; quadrant q owns columns
        # q*qs*D..): one start=True opener per group clears the bank, all
        # other matmuls accumulate disjoint regions. Per-group start/stop
        # lets the flush/recombine/projection pipeline run mid-stream
        # instead of trailing the whole x stream.
        QSMAX = max(qs for _, qs in GROUPS)
        sums_g = [bpool.tile([GPW, nquad * QSMAX * D], BF16, name=f"sums{g}")
                  for g in range(len(GROUPS))]
        tmp = bpool.tile([P, len(GROUPS) * O * QSMAX * D], F32)
        proj = bpool.tile([P, nstripe * O], F32)
        outv = bpool.tile([P, nstripe * O], F32)

        for g, (s0, qs) in enumerate(GROUPS):
            qsum = ppool.tile([GPW, nquad * QSMAX * D], F32,
                              tag="qsum", name=f"qsum{g}")
            opener = None
            last_mm = None
            for s in range(s0, s0 + qs):
                for q in range(nquad):
                    col = (q * qs + (s - s0)) * D
                    mms = []
                    for j in range(CPW):
                        m = (s * wps + q) * CPW + j
                        ml = m - s * cps
                        mms.append(nc.tensor.matmul(
                            out=qsum[:, col : col + D],
                            lhsT=tap(onehot, s * cps * GPW + ml,
                                     [onehot.ap[0], [cps, GPW]]),
                            rhs=B[:, m * D : (m + 1) * D],
                            start=(opener is None and not mms),
                            stop=False,
                            tile_position=(0, 0),
                            skip_group_check=True,
                        ))
                    glast = (s == s0 + qs - 1) and (q == nquad - 1)
                    for oc in range(plan.ocps):
                        ch = s * plan.ocps + oc
                        mms.append(nc.tensor.matmul(
                            out=qsum[:, col : col + D],
                            lhsT=tap(oneO, ch + q * GPW * noch,
                                     [[oneO.ap[0][0], osb], [noch, GPW]]),
                            rhs=Bo[:osb, ch * D : (ch + 1) * D],
                            start=False,
                            stop=(glast and oc == plan.ocps - 1),
                            tile_position=(0, 0),
                            skip_group_check=True,
                        ))
                    if opener is None:
                        opener = mms[0]
                        mms = mms[1:]
                    for mm in mms:
                        tile.add_dep_helper(mm.ins, opener.ins, sync=False,
                                            reason="psum opener first")
                    last_mm = mms[-1] if mms else opener

            # flush: PSUM f32 -> SBUF bf16 (1/count is in winv). The last
            # group flushes on DVE (idle at that point) to cut a cross-
            # engine hop off the tail; earlier groups use the idle ACT.
            if g >= len(GROUPS) - 2:
                nc.vector.tensor_copy(
                    out=sums_g[g][:, : nquad * qs * D],
                    in_=qsum[:, : nquad * qs * D],
                )
            else:
                nc.scalar.copy(
                    out=sums_g[g][:, : nquad * qs * D],
                    in_=qsum[:, : nquad * qs * D],
                )

            # recombine quadrants -> pall [P, qs*D] (PE, bf16 exact)
            pall = ppool.tile([P, QSMAX * D], F32, tag="pall", name=f"pall{g}")
            for q in range(nquad):
                nc.tensor.matmul(
                    out=pall[:, : qs * D],
                    lhsT=i4_sb[:, q * P : (q + 1) * P],
                    rhs=sums_g[g][:, q * qs * D : (q + 1) * qs * D],
                    start=(q == 0),
                    stop=(q == nquad - 1),
                    tile_position=(0, 0),
                    skip_group_check=True,
                )
            # projection: tmp[p,o,s,f] = mean * W[o,f]*inv; reduce f; + bias
            lc = (s0 + qs) * cps - 1     # last chunk this group consumes
            ci = 0
            for ti2, kc2 in enumerate(KCS):
                ci += kc2
                if lc < ci:
                    gate = reds[ti2]
                    break
            toff = g * O * QSMAX * D
            tmp_tt = nc.vector.tensor_tensor(
                out=tap(tmp, toff, [tmp.ap[0], [qs * D, O], [D, qs], [1, D]]),
                in0=tap(pall, 0, [pall.ap[0], [0, O], [D, qs], [1, D]]),
                in1=tap(winv_sb, s0 * D,
                        [winv_sb.ap[0], [nstripe * D, O], [D, qs], [1, D]]),
                op=mybir.AluOpType.mult,
            )
            tile.add_dep_helper(tmp_tt.ins, gate.ins, sync=False,
                                reason="slot group chain at data readiness")
            nc.vector.tensor_reduce(
                out=tap(proj, s0 * O, [proj.ap[0], [1, O], [O, qs]]),
                in_=tap(tmp, toff, [tmp.ap[0], [qs * D, O], [D, qs], [1, D]]),
                axis=mybir.AxisListType.X,
                op=mybir.AluOpType.add,
            )
            nc.vector.tensor_tensor(
                out=outv[:, s0 * O : (s0 + qs) * O],
                in0=proj[:, s0 * O : (s0 + qs) * O],
                in1=bne_sb[:, s0 * O : (s0 + qs) * O],
                op=mybir.AluOpType.add,
            )
            (nc.sync if g % 2 == 0 else nc.scalar).dma_start(
                out=ap_of(out_t, s0 * P * O, [[O, P], [P * O, qs], [1, O]]),
                in_=outv[:, s0 * O : (s0 + qs) * O],
            )

    nc.compile()
    return nc


# ----------------------------------------------------------------------------
# entry point
# ----------------------------------------------------------------------------

_CACHE = {}
_LAST_RESULTS = None


def kernel(x, batch, W, b):
    global _LAST_RESULTS
    x = np.asarray(x, dtype=np.float32)
    batch = np.asarray(batch)
    W = np.asarray(W, dtype=np.float32)
    b = np.asarray(b, dtype=np.float32)

    n_cores = 8
    G = 16384
    plan = Plan(batch, n_cores, G, W, b)

    key = (plan.lslots, plan.nchunk, plan.noch, plan.osps)
    if key not in _CACHE:
        _CACHE[key] = build_program(plan, W, b)
    nc = _CACHE[key]

    in_maps = [plan.core_tables(c, x) for c in range(n_cores)]

    def _run():
        return run_bass_kernel_spmd(nc, in_maps, core_ids=list(range(n_cores)))

    try:
        res = _run()
    except ModuleNotFoundError:
        # BASS_TRACE was set but this container lacks the axon NTFF profiling
        # hook (antenv.axon_hooks) — retry with tracing disabled.
        import os
        os.environ["BASS_NEVER_TRACE"] = "1"
        res = _run()
    except Exception as e:  # transient device/terminal failure -> one retry
        if not any(k in str(e) for k in ("UNAVAILABLE", "UNRECOVERABLE")):
            raise
        import time as _time
        _time.sleep(10.0)
        res = _run()
    _LAST_RESULTS = res
    out = np.concatenate(
        [res.results[c]["out"].reshape(plan.gpc, O) for c in range(n_cores)],
        axis=0,
    )
    return out.astype(np.float32)


if __name__ == "__main__":
    # tiny smoke test of the planner only
    rng = np.random.default_rng(0)
    N, G = 400_000, 16384
    batch = np.sort(rng.integers(0, G, N))
    x = rng.standard_normal((N, D), dtype=np.float32)
    plan = Plan(batch, 8, G)
    print("lslots", plan.lslots, "nchunk", plan.nchunk, "osps", plan.osps)
    t = plan.core_tables(0, x)
    for k, v in t.items():
        print(k, v.shape, v.dtype)


# revision 32
# speedup vs baseline: 1.0162x; 1.0162x over previous
"""Trainium2 Bass kernel for fused segment-mean + linear projection.

Reference computation (for x[N,15], sorted batch[N] in [0,G), W[5,15], b[5]):
    sums[g]  = segment_sum(x, batch)          # [G, 15]
    counts[g]= segment_sum(1, batch)          # [G]
    mean     = sums / max(counts, 1)
    out      = where(counts > 0, mean @ W.T + b, 0)   # [G, 5]

Strategy (8 NeuronCores, data parallel over contiguous graph-id ranges):
  Host (index-only preprocessing; x is repacked and cast to bf16 --
  the 2e-2 correctness budget dwarfs bf16 quantization error, and it
  halves the HBM traffic this memory-bound kernel is limited by):
    - each core owns G/8 consecutive graphs; its nodes are repacked into
      "windows" of GPW=32 graphs x 8192 node-slots (4 chunks of 128
      16-node blocks), each graph zero-padded to a 16-node multiple so
      every block belongs to exactly one graph. Graphs that do not fit
      their window spill whole into a small per-stripe overflow stream.
    - the packed stream is bf16 PARTITION-MAJOR: each SBUF partition's
      chunk range is contiguous in DRAM, so DMA descriptors stay >= 512B
      (full 360 GB/s; no small-descriptor penalty) at half the bytes.
    - all program shapes / the matmul schedule are data-independent, so
      one SPMD program serves all 8 cores; per-core data differs only in
      the input tables.
  Device (per core):
    - DMA bf16 x tiles (tapered sizes); block sums via a 4-stage DVE
      tensor_tensor halving tree (16->8->4->2->1 nodes). Blocks are
      node-major so every stage adds two contiguous 15-element runs:
      all operands are 2-byte stride-1 -> DVE 2x mode on every stage
      (tensor_reduce has no fast modes, hence the tt tree).
    - PE bf16 matmuls with one-hot matrices scatter-add block sums
      into per-stripe-group PSUM accumulators (f32, exact); the one-hot
      arenas are built by GPSIMD local_scatter from host-made int16
      index tables (zero DVE cost); per-group start/stop lets each
      group flush mid-stream.
    - per group: one copy flushes PSUM -> bf16 (ACT mid-stream; DVE
      for the last two groups, cutting cross-engine hops off the tail),
      a PE identity matmul recombines the four 32-graph quadrants into
      128 rows, and DVE applies the fused winv = W * (1/count) table,
      reduces over features, adds bias * nonempty -> out rows; the
      1-stripe final group keeps the post-stream tail short.
  Host: concatenate the 8 core outputs.
"""

import sys

for _p in ("/opt/trn_rl_repo",):
    if _p not in sys.path:
        sys.path.insert(0, _p)

import numpy as np
import ml_dtypes
from contextlib import ExitStack

import concourse.bass as bass
import concourse.bacc as bacc
import concourse.tile as tile
from concourse import mybir
from concourse.bass_utils import run_bass_kernel_spmd

P = 128          # partitions
BLK = 16         # nodes per block
D = 15           # feature dim
O = 5            # output dim
GPW = 32         # graphs per window
CPW = 4          # chunks per window (chunk = 128 blocks = 2048 node slots)
SLOTS_W = CPW * P * BLK  # node slots per window

F32 = mybir.dt.float32
BF16 = mybir.dt.bfloat16
BNP = ml_dtypes.bfloat16


# ----------------------------------------------------------------------------
# host planner
# ----------------------------------------------------------------------------

class Plan:
    """Per-run packing plan. All *shape* fields are uniform across cores."""

    def __init__(self, batch, n_cores, G, W=None, b=None):
        self.W = (np.zeros((O, D), np.float32) if W is None
                  else np.asarray(W, np.float32))
        self.b = (np.zeros(O, np.float32) if b is None
                  else np.asarray(b, np.float32))
        batch = np.asarray(batch)
        N = batch.shape[0]
        assert G % (n_cores * P) == 0
        self.G = G
        self.n_cores = n_cores
        self.gpc = G // n_cores                  # graphs per core
        self.nwin = self.gpc // GPW              # windows per core
        self.nstripe = self.gpc // P             # psum stripes per core
        self.nchunk = self.nwin * CPW            # main-stream chunks per core
        self.lslots = self.nwin * SLOTS_W        # node slots per core
        assert self.nwin % self.nstripe == 0
        self.wps = self.nwin // self.nstripe     # windows per stripe
        self.cps = self.nchunk // self.nstripe   # chunks per stripe

        bounds = np.searchsorted(batch, np.arange(G + 1))
        counts = np.diff(bounds).astype(np.int64)
        self.counts = counts
        self.inv = (1.0 / np.maximum(counts, 1.0)).astype(np.float32)
        self.nonempty = (counts > 0).astype(np.float32)

        nblk_g = (counts + BLK - 1) // BLK       # blocks per graph

        # ---- window placement (per core) ----
        self.placements = []
        self.overflow = []
        max_oslots = 8 * BLK
        for c in range(n_cores):
            g0 = c * self.gpc
            placed = []
            oflow = [[] for _ in range(self.nstripe)]
            for w in range(self.nwin):
                gs = [g0 + w * GPW + j for j in range(GPW)
                      if nblk_g[g0 + w * GPW + j] > 0]
                need = {g: int(nblk_g[g]) * BLK for g in gs}
                over = sum(need.values()) - SLOTS_W
                spill = []
                while over > 0:
                    # spill the smallest graph that covers the overage, or
                    # the largest graph if none does (minimizes spilled slots)
                    cand = [g for g in gs if need[g] >= over]
                    g = (min(cand, key=need.get) if cand
                         else max(gs, key=need.get))
                    gs.remove(g); spill.append(g); over -= need[g]
                pos = 0
                for g in gs:
                    placed.append((g, w * SLOTS_W + pos))
                    pos += need[g]
                oflow[w // self.wps].extend(spill)
            for s in range(self.nstripe):
                used = sum(int(nblk_g[g]) * BLK for g in oflow[s])
                max_oslots = max(max_oslots, used)
            self.placements.append(placed)
            self.overflow.append(oflow)

        # overflow blocks per stripe: a single partial chunk (K<128 matmuls)
        # when it fits, whole chunks otherwise
        max_oblk = -(-max_oslots // BLK)
        if max_oblk <= P:
            self.osb = max(8, -(-max_oblk // 8) * 8)
            self.ocps = 1
        else:
            self.osb = P
            self.ocps = -(-max_oblk // P)
        self.osps = self.ocps * self.osb * BLK   # overflow slots per stripe
        self.noch = self.nstripe * self.ocps     # total overflow chunks
        self.bounds = bounds
        self.N = N

    def core_tables(self, c, x):
        """Build per-core input arrays. x is the full [N, D] float32 array."""
        lslots, nchunk = self.lslots, self.nchunk
        g0 = c * self.gpc

        idx = np.full(lslots, -1, dtype=np.int64)
        asg = np.full(lslots // BLK, -1.0, dtype=np.float32)  # local graph/blk
        for g, base in self.placements[c]:
            s0, cnt = int(self.bounds[g]), int(self.counts[g])
            idx[base : base + cnt] = np.arange(s0, s0 + cnt)
            nb = (cnt + BLK - 1) // BLK
            asg[base // BLK : base // BLK + nb] = g - g0

        # overflow stream: per stripe a fixed region of osps slots
        oidx = np.full(self.nstripe * self.osps, -1, dtype=np.int64)
        oasg = np.full(self.nstripe * self.osps // BLK, -1.0, dtype=np.float32)
        for s in range(self.nstripe):
            pos = s * self.osps
            for g in self.overflow[c][s]:
                s0, cnt = int(self.bounds[g]), int(self.counts[g])
                nb = (cnt + BLK - 1) // BLK
                assert pos + nb * BLK <= (s + 1) * self.osps, "overflow overrun"
                oidx[pos : pos + cnt] = np.arange(s0, s0 + cnt)
                oasg[pos // BLK : pos // BLK + nb] = (g - g0) - s * P
                pos += nb * BLK

        def pack(idx_arr, nch, parts):
            # gather, then block t -> (partition t % parts, chunk t // parts),
            # partition-major layout, blocks in natural node-major order
            # (so every halving-tree stage is stride-1 innermost -> DVE 2x)
            out = x[np.clip(idx_arr, 0, self.N - 1)]
            out[idx_arr < 0] = 0.0
            out = out.reshape(nch, parts, BLK * D).transpose(1, 0, 2)
            return np.ascontiguousarray(out.reshape(parts, nch * BLK * D)
                                        .astype(BNP))

        xw = pack(idx, nchunk, P)                 # [P, nchunk*CB] bf16
        xb = pack(oidx, self.noch, self.osb)      # [osb, noch*CB] bf16

        # block t -> partition t%128, chunk t//128; window of chunk m = m//CPW
        # idxJ[p, m]: scatter index into the one-hot arena (w-major per
        # stripe, 1024-column sections of 2 stripes), -1 for padding blocks
        t = np.arange(lslots // BLK)
        win_base = (t // P // CPW) * GPW
        w = np.where(asg >= 0, asg - win_base, -1.0).astype(np.int64)
        m = t // P
        s = m // self.cps
        iJ = np.where(w >= 0,
                      (s % 2) * (self.cps * GPW) + w * self.cps + m % self.cps,
                      -1)
        idxJ = np.full((P, nchunk), -1, dtype=np.int16)
        idxJ[t % P, m] = iJ.astype(np.int16)

        # idxO[p, sec*noch + ch]: index into the overflow one-hot arena
        # (w*noch + ch layout, 1024-column sections of wspan=1024//noch)
        oasgT = np.full((P, self.noch), -1, dtype=np.int64)
        oasgT[: self.osb, :] = oasg.reshape(self.noch, self.osb).T.astype(
            np.int64)
        wspan = 1024 // self.noch
        nsec = P // wspan
        ch = np.arange(self.noch)[None, :]
        idxO = np.full((P, nsec * self.noch), -1, dtype=np.int16)
        for sec in range(nsec):
            wl = oasgT - sec * wspan
            ok = (wl >= 0) & (wl < wspan)
            idxO[:, sec * self.noch : (sec + 1) * self.noch] = np.where(
                ok, wl * self.noch + ch, -1).astype(np.int16)

        def stripe_pack(v):
            # graph g (local) -> [partition g%128, col g//128]
            return np.ascontiguousarray(
                v[g0 : g0 + self.gpc].reshape(self.nstripe, P).T.astype(np.float32)
            )

        inv_ps = stripe_pack(self.inv)                       # [P, nstripe] f32
        # fused W x 1/count table: winv[p, o, s, f] = W[o, f] * inv[g(p, s)]
        winv = (inv_ps[:, None, :, None] *
                self.W[None, :, None, :]).astype(BNP)        # [P,O,S,D] bf16
        bne = (stripe_pack(self.nonempty)[:, :, None] *
               self.b[None, None, :]).reshape(P, self.nstripe * O)
        # pad bne to 128 f32 cols so the DMA descriptor is >= 512B
        bne_pad = np.zeros((P, 128), np.float32)
        bne_pad[:, : self.nstripe * O] = bne
        # identity selection matrix for the quadrant recombine:
        # i4[k, q*P + m] = 1.0 iff m == q*GPW + k   (bf16, built on host)
        nq = P // GPW
        i4 = np.zeros((GPW, nq * P), BNP)
        for q in range(nq):
            for k in range(GPW):
                i4[k, q * P + q * GPW + k] = 1.0

        return {
            "xw": xw.reshape(-1),
            "xb": xb.reshape(-1),
            "idxJ": idxJ,
            "idxO": idxO,
            "winv": np.ascontiguousarray(winv.reshape(P, -1)),
            "bne": bne_pad,
            "i4": i4,
        }


# ----------------------------------------------------------------------------
# device program
# ----------------------------------------------------------------------------

def build_program(plan, W, b):
    """Build + compile the SPMD Bass program (one program, 8 cores)."""
    nchunk, noch, nstripe = plan.nchunk, plan.noch, plan.nstripe
    cps, wps, osb = plan.cps, plan.wps, plan.osb
    CB = BLK * D     # elements per block (240)

    nc = bacc.Bacc("TRN2", target_bir_lowering=False, debug=False)

    xw = nc.dram_tensor("xw", [P * nchunk * CB], BF16, kind="ExternalInput")
    xb = nc.dram_tensor("xb", [osb * noch * CB], BF16, kind="ExternalInput")
    nsec_o = P // (1024 // noch)
    idxJ = nc.dram_tensor("idxJ", [P, nchunk], mybir.dt.int16,
                          kind="ExternalInput")
    idxO = nc.dram_tensor("idxO", [P, nsec_o * noch], mybir.dt.int16,
                          kind="ExternalInput")
    winv_t = nc.dram_tensor("winv", [P, O * nstripe * D], BF16,
                            kind="ExternalInput")
    bne_t = nc.dram_tensor("bne", [P, 128], F32, kind="ExternalInput")
    i4_t = nc.dram_tensor("i4", [GPW, (P // GPW) * P], BF16,
                          kind="ExternalInput")
    out_t = nc.dram_tensor("out", [plan.gpc * O], F32, kind="ExternalOutput")

    # x tiles: chunks per DMA tile. Tapered: small first tiles so DVE starts
    # early and is never starved, small last tiles so the post-DMA tail is
    # short. Stage-1 of the big middle tiles runs on the (otherwise idle)
    # GPSIMD engine to keep the DVE chain under the DMA roofline.
    KCS = []
    rem = nchunk
    TAIL = (16, 12, 8, 4, 2)
    for k in (2, 4, 8, 8, 16, 16):
        kc = min(k, rem)
        if kc:
            KCS.append(kc); rem -= kc
    while rem > sum(TAIL):
        KCS.append(min(32, rem - sum(TAIL))); rem -= KCS[-1]
    for k in TAIL:
        kc = min(k, rem)
        if kc:
            KCS.append(kc); rem -= kc
    assert sum(KCS) == nchunk and rem == 0
    nquad = P // GPW
    # flush/recombine/projection groups of stripes; the trailing 3+1 split
    # keeps the very last group (one stripe) tiny so the tail is short.
    GROUPS = []
    s = 0
    while s + 4 < nstripe:
        GROUPS.append((s, 4)); s += 4
    if nstripe - s > 1:
        GROUPS.append((s, nstripe - s - 1)); s = nstripe - 1
    GROUPS.append((s, 1))

    with tile.TileContext(nc) as tc, ExitStack() as ctx:
        consts = ctx.enter_context(tc.tile_pool(name="consts", bufs=1))
        xpool = ctx.enter_context(tc.tile_pool(name="xpool", bufs=5))
        spool = ctx.enter_context(tc.tile_pool(name="spool", bufs=2))
        bpool = ctx.enter_context(tc.tile_pool(name="bpool", bufs=1))
        ppool = ctx.enter_context(tc.tile_pool(name="ppool", bufs=2, space="PSUM"))

        def ap_of(handle, offset, pattern):
            return bass.AP(tensor=handle.ap().tensor, offset=offset, ap=pattern)

        def tap(t, offset, pattern):
            return bass.AP(tensor=t.tensor, offset=t.offset + offset, ap=pattern)

        # ---- constant tables (ACT HWDGE ring; keeps SP ring free for x) ----
        # Only asgJ is needed immediately (first one-hot builds); the rest is
        # pinned behind the early x tiles so the x stream owns the DMA bus.
        idxJ_sb = consts.tile([P, nchunk], mybir.dt.int16)
        idxO_sb = consts.tile([P, nsec_o * noch], mybir.dt.int16)
        i4_sb = consts.tile([GPW, nquad * P], BF16)
        winv_sb = consts.tile([P, O * nstripe * D], BF16)
        bne_sb = consts.tile([P, 128], F32)

        # all-ones data row for the one-hot scatters
        ones_sb = consts.tile([P, cps * 2], BF16)
        nc.gpsimd.memset(ones_sb[:], 1.0)

        # one-hot arenas, built by GPSIMD local_scatter (dst is zeroed by
        # the instruction itself; negative indices = padding are ignored):
        #   onehot[p, s*cps*GPW + w*cps + ml] = 1 at idxJ positions
        #   oneO[p, w*noch + ch]              = 1 at idxO positions
        onehot = bpool.tile([P, nchunk * GPW], BF16)
        oneO = bpool.tile([P, P * noch], BF16)
        SEC = 2 * cps * GPW          # columns per scatter section (2 stripes)
        assert SEC == 1024 and SEC * 32 < 2 ** 16

        def emit_oh(sec):
            nix = 2 * cps            # chunks per section
            return nc.gpsimd.local_scatter(
                out_ap=onehot[:, sec * SEC : (sec + 1) * SEC],
                data_ap=ones_sb[:, :nix],
                idxs_ap=idxJ_sb[:, sec * nix : (sec + 1) * nix],
                channels=P,
                num_elems=SEC,
                num_idxs=nix,
            )

        def emit_oo(sec):
            return nc.gpsimd.local_scatter(
                out_ap=oneO[:, sec * 1024 : (sec + 1) * 1024],
                data_ap=ones_sb[:, :noch],
                idxs_ap=idxO_sb[:, sec * noch : (sec + 1) * noch],
                channels=P,
                num_elems=1024,
                num_idxs=noch,
            )

        # ---- block-sum halving tree (DVE tensor_tensor, 2x mode) ----
        # src layout per chunk-column: [D, W] feature-major, W nodes.
        def emit_tree(src, soff, dst, doff, kc, parts, tag):
            # node-major halving tree: every stage adds two contiguous
            # 15-element runs (stride-1 innermost on all operands -> 2x)
            cur, co, w = src, soff, BLK
            last = None
            while w > 1:
                h = w // 2
                if h > 1:
                    nxt = spool.tile([P, kc * D * h], BF16,
                                     tag=f"{tag}{h}", name=f"{tag}{h}")
                    no = 0
                else:
                    nxt, no = dst, doff
                cp = [cur.ap[0][0], parts]
                last = nc.vector.tensor_tensor(
                    out=tap(nxt, no, [[nxt.ap[0][0], parts],
                                      [D * h, kc], [D, h], [1, D]]),
                    in0=tap(cur, co, [cp, [D * w, kc], [D, h], [1, D]]),
                    in1=tap(cur, co + h * D,
                            [cp, [D * w, kc], [D, h], [1, D]]),
                    op=mybir.AluOpType.add,
                )
                cur, co, w = nxt, no, h
            return last

        # ---- overflow stream ----
        xb_sb = bpool.tile([P, noch * CB], BF16)
        Bo = bpool.tile([P, noch * D], BF16)

        # ---- main stream: tapered tiles -> block sums B ----
        B = bpool.tile([P, nchunk * D], BF16)
        KCMAX = max(KCS)
        c0 = 0
        oh_next = 0
        reds = []
        dmas = []
        for ti, KC in enumerate(KCS):
            xt = xpool.tile([P, KCMAX * CB], BF16, tag="xt", name="xt")
            # the first small tiles alternate between the SP and ACT HWDGE
            # rings so their DMA setup latencies overlap and the bus stays
            # packed from the start
            eng = nc.scalar if ti in (1, 3) else nc.sync
            xdma = eng.dma_start(
                out=xt[:, : KC * CB],
                in_=ap_of(xw, c0 * CB, [[nchunk * CB, P], [1, KC * CB]]),
            )
            dmas.append(xdma)
            red = emit_tree(xt, 0, B, c0 * D, KC, P, "st")
            c0 += KC
            reds.append(red)
            # secondary table loads, pinned behind the early x tiles so the
            # x stream owns the DMA bus at the start
            if ti == 0:
                # index tables ride the Pool ring (their consumer, the
                # local_scatter, runs there anyway); xb on ACT
                for dma in (
                    nc.gpsimd.dma_start(out=idxJ_sb[:], in_=idxJ.ap()),
                    nc.gpsimd.dma_start(out=idxO_sb[:], in_=idxO.ap()),
                    nc.scalar.dma_start(
                        out=xb_sb[:osb, :],
                        in_=ap_of(xb, 0, [[noch * CB, osb], [1, noch * CB]]),
                    ),
                ):
                    tile.add_dep_helper(dma.ins, dmas[0].ins, sync=False,
                                        reason="table loads behind x tiles")
            if ti == 3:
                for dma in (
                    nc.scalar.dma_start(out=i4_sb[:], in_=i4_t.ap()),
                    nc.scalar.dma_start(out=winv_sb[:], in_=winv_t.ap()),
                    nc.scalar.dma_start(out=bne_sb[:], in_=bne_t.ap()),
                ):
                    tile.add_dep_helper(dma.ins, dmas[3].ins, sync=False,
                                        reason="table loads behind x tiles")
            # Pack the small side-jobs behind the early tiles' tree work so
            # the scheduler cannot hoist them ahead of the x pipeline.
            if ti == 0:
                for sec in range(nstripe // 2):
                    emit_oh(sec)
                for sec in range(nsec_o):
                    emit_oo(sec)
            if ti == min(2, len(KCS) - 1):
                bo = emit_tree(xb_sb, 0, Bo, 0, noch, osb, "ob")
                tile.add_dep_helper(bo.ins, red.ins, sync=False,
                                    reason="Bo tree in early DVE idle")

        # ---- routing matmuls, grouped by stripe-group ----
        # Each group accumulates its stripes into ONE combined quadrant PSUM
        # tile [GPW, nquad*qs*D] (single bank; quadrant q owns columns
        # q*qs*D..): one start=True opener per group clears the bank, all
        # other matmuls accumulate disjoint regions. Per-group start/stop
        # lets the flush/recombine/projection pipeline run mid-stream
        # instead of trailing the whole x stream.
        QSMAX = max(qs for _, qs in GROUPS)
        sums_g = [bpool.tile([GPW, nquad * QSMAX * D], BF16, name=f"sums{g}")
                  for g in range(len(GROUPS))]
        tmp = bpool.tile([P, len(GROUPS) * O * QSMAX * D], F32)
        proj = bpool.tile([P, nstripe * O], F32)
        outv = bpool.tile([P, nstripe * O], F32)

        for g, (s0, qs) in enumerate(GROUPS):
            qsum = ppool.tile([GPW, nquad * QSMAX * D], F32,
                              tag="qsum", name=f"qsum{g}")
            opener = None
            last_mm = None
            for s in range(s0, s0 + qs):
                for q in range(nquad):
                    col = (q * qs + (s - s0)) * D
                    mms = []
                    for j in range(CPW):
                        m = (s * wps + q) * CPW + j
                        ml = m - s * cps
                        mms.append(nc.tensor.matmul(
                            out=qsum[:, col : col + D],
                            lhsT=tap(onehot, s * cps * GPW + ml,
                                     [onehot.ap[0], [cps, GPW]]),
                            rhs=B[:, m * D : (m + 1) * D],
                            start=(opener is None and not mms),
                            stop=False,
                            tile_position=(0, 0),
                            skip_group_check=True,
                        ))
                    glast = (s == s0 + qs - 1) and (q == nquad - 1)
                    for oc in range(plan.ocps):
                        ch = s * plan.ocps + oc
                        mms.append(nc.tensor.matmul(
                            out=qsum[:, col : col + D],
                            lhsT=tap(oneO, ch + q * GPW * noch,
                                     [[oneO.ap[0][0], osb], [noch, GPW]]),
                            rhs=Bo[:osb, ch * D : (ch + 1) * D],
                            start=False,
                            stop=(glast and oc == plan.ocps - 1),
                            tile_position=(0, 0),
                            skip_group_check=True,
                        ))
                    if opener is None:
                        opener = mms[0]
                        mms = mms[1:]
                    for mm in mms:
                        tile.add_dep_helper(mm.ins, opener.ins, sync=False,
                                            reason="psum opener first")
                    last_mm = mms[-1] if mms else opener

            # flush: PSUM f32 -> SBUF bf16 (1/count is in winv). The last
            # group flushes on DVE (idle at that point) to cut a cross-
            # engine hop off the tail; earlier groups use the idle ACT.
            if g >= len(GROUPS) - 2:
                nc.vector.tensor_copy(
                    out=sums_g[g][:, : nquad * qs * D],
                    in_=qsum[:, : nquad * qs * D],
                )
            else:
                nc.scalar.copy(
                    out=sums_g[g][:, : nquad * qs * D],
                    in_=qsum[:, : nquad * qs * D],
                )

            # recombine quadrants -> pall [P, qs*D] (PE, bf16 exact)
            pall = ppool.tile([P, QSMAX * D], F32, tag="pall", name=f"pall{g}")
            for q in range(nquad):
                nc.tensor.matmul(
                    out=pall[:, : qs * D],
                    lhsT=i4_sb[:, q * P : (q + 1) * P],
                    rhs=sums_g[g][:, q * qs * D : (q + 1) * qs * D],
                    start=(q == 0),
                    stop=(q == nquad - 1),
                    tile_position=(0, 0),
                    skip_group_check=True,
                )
            # projection: tmp[p,o,s,f] = mean * W[o,f]*inv; reduce f; + bias
            lc = (s0 + qs) * cps - 1     # last chunk this group consumes
            ci = 0
            for ti2, kc2 in enumerate(KCS):
                ci += kc2
                if lc < ci:
                    gate = reds[ti2]
                    break
            toff = g * O * QSMAX * D
            tmp_tt = nc.vector.tensor_tensor(
                out=tap(tmp, toff, [tmp.ap[0], [qs * D, O], [D, qs], [1, D]]),
                in0=tap(pall, 0, [pall.ap[0], [0, O], [D, qs], [1, D]]),
                in1=tap(winv_sb, s0 * D,
                        [winv_sb.ap[0], [nstripe * D, O], [D, qs], [1, D]]),
                op=mybir.AluOpType.mult,
            )
            tile.add_dep_helper(tmp_tt.ins, gate.ins, sync=False,
                                reason="slot group chain at data readiness")
            nc.vector.tensor_reduce(
                out=tap(proj, s0 * O, [proj.ap[0], [1, O], [O, qs]]),
                in_=tap(tmp, toff, [tmp.ap[0], [qs * D, O], [D, qs], [1, D]]),
                axis=mybir.AxisListType.X,
                op=mybir.AluOpType.add,
            )
            nc.vector.tensor_tensor(
                out=outv[:, s0 * O : (s0 + qs) * O],
                in0=proj[:, s0 * O : (s0 + qs) * O],
                in1=bne_sb[:, s0 * O : (s0 + qs) * O],
                op=mybir.AluOpType.add,
            )
            (nc.sync if g % 2 == 0 else nc.scalar).dma_start(
                out=ap_of(out_t, s0 * P * O, [[O, P], [P * O, qs], [1, O]]),
                in_=outv[:, s0 * O : (s0 + qs) * O],
            )

    nc.compile()
    return nc


# ----------------------------------------------------------------------------
# entry point
# ----------------------------------------------------------------------------

_CACHE = {}
_LAST_RESULTS = None


def kernel(x, batch, W, b):
    global _LAST_RESULTS
    x = np.asarray(x, dtype=np.float32)
    batch = np.asarray(batch)
    W = np.asarray(W, dtype=np.float32)
    b = np.asarray(b, dtype=np.float32)

    n_cores = 8
    G = 16384
    plan = Plan(batch, n_cores, G, W, b)

    key = (plan.lslots, plan.nchunk, plan.noch, plan.osps)
    if key not in _CACHE:
        _CACHE[key] = build_program(plan, W, b)
    nc = _CACHE[key]

    in_maps = [plan.core_tables(c, x) for c in range(n_cores)]

    def _run():
        return run_bass_kernel_spmd(nc, in_maps, core_ids=list(range(n_cores)))

    try:
        res = _run()
    except ModuleNotFoundError:
        # BASS_TRACE was set but this container lacks the axon NTFF profiling
        # hook (antenv.axon_hooks) — retry with tracing disabled.
        import os
        os.environ["BASS_NEVER_TRACE"] = "1"
        res = _run()
    except Exception as e:  # transient device/terminal failure -> one retry
        if not any(k in str(e) for k in ("UNAVAILABLE", "UNRECOVERABLE")):
            raise
        import time as _time
        _time.sleep(10.0)
        res = _run()
    _LAST_RESULTS = res
    out = np.concatenate(
        [res.results[c]["out"].reshape(plan.gpc, O) for c in range(n_cores)],
        axis=0,
    )
    return out.astype(np.float32)


if __name__ == "__main__":
    # tiny smoke test of the planner only
    rng = np.random.default_rng(0)
    N, G = 400_000, 16384
    batch = np.sort(rng.integers(0, G, N))
    x = rng.standard_normal((N, D), dtype=np.float32)
    plan = Plan(batch, 8, G)
    print("lslots", plan.lslots, "nchunk", plan.nchunk, "osps", plan.osps)
    t = plan.core_tables(0, x)
    for k, v in t.items():
        print(k, v.shape, v.dtype)
